# revision 21
# baseline (speedup 1.0000x reference)
"""Involution kernel for Trainium2, 8-core data-parallel (1 batch image per core).

Reference computation (per image, NHWC, C=64, G=4 groups, K=3, reduction 4):
    t    = relu(BN(x @ w1 + b1))            # [H,W,16]
    kern = t @ w2 + b2                      # [H,W,36], e = (ki*3+kj)*4 + g
    out[h,w,c] = sum_p kern[h,w, 4p + c%4] * xpad[h+di-1, w+dj-1, c]

Device strategy (v2):
  * 256 subtiles of 12x12 interior, each in one SBUF partition's free dim
    with a 2-ring halo (16x16x64 window, x2 layout).  3x3 taps are free-dim
    offsets.  2 blocks of 128 subtiles.
  * kern is computed ONLY for the 144 interior positions per subtile.
  * mm1 (x @ w1bn) runs as 4 concurrent column-group matmuls
    (tile_position stripes at PSUM partitions 0/32/64/96, 17 rows each:
    16 bottleneck channels + a ones row for the b2 path), on a q-major
    pixel layout (col = q*128 + st).  Scalar engine applies ReLU while
    evacuating [128,512] PSUM tiles -> tp.
  * mm2: per-position matmul lhsT=tp-slab [17,128st], rhs=w2 replica at the
    matching partition base; out [128st, 36].  Scalar engine casts to bf16.
  * Involution: DVE computes the 9 per-tap products (bf16, 2x mode);
    the 8 adds run on the TENSOR engine as identity matmuls accumulating
    in PSUM (9 matmuls per 512-col bank group).  Scalar engine evacuates
    f32 PSUM -> bf16 out tile; DMA writes interior layout directly.
"""

import numpy as np
import ml_dtypes

import concourse.bass as bass
import concourse.bacc as bacc
import concourse.mybir as mybir
from concourse.tile import TileContext
from concourse.bass_utils import run_bass_kernel_spmd

BF16 = mybir.dt.bfloat16
F32 = mybir.dt.float32
NPF32 = np.float32
NPBF16 = ml_dtypes.bfloat16

B, H, W, C = 8, 192, 192, 64
G, K, CR, E = 4, 3, 16, 36
BN_EPS = 1e-3
S = 12                  # subtile interior
S2 = S + 2              # 1-ring padded subtile size (14)
NG = H // S             # 16 subtiles per axis
NST = NG * NG           # 256 subtiles
NB = 2                  # partition blocks of 128 subtiles
Q = S * S               # 144 interior positions per subtile
F2 = S2 * S2 * C        # 16384 x2 free elems per subtile
FO = Q * C              # 9216 out free elems per subtile
NPB = 128 * Q           # 18432 pixels per block
WCH = 1536              # MAC chunk width (2 interior rows = 24 q)
NCH = FO // WCH         # 6 chunks per block

_CACHE = {}


def _build_program():
    if "nc" in _CACHE:
        return _CACHE["nc"]
    nc = bacc.Bacc(None, target_bir_lowering=False)
    x2_d = nc.dram_tensor("x2", [NST, F2], BF16, kind="ExternalInput")
    xtq_d = nc.dram_tensor("xtq", [C + 1, NB * NPB], BF16, kind="ExternalInput")
    w1_d = nc.dram_tensor("w1b", [C + 1, 32], BF16, kind="ExternalInput")
    w2_d = nc.dram_tensor("w2r", [128, E], BF16, kind="ExternalInput")
    id_d = nc.dram_tensor("idn", [128, 128], BF16, kind="ExternalInput")
    o_d = nc.dram_tensor("o", [NST, FO], BF16, kind="ExternalOutput")

    RELU = mybir.ActivationFunctionType.Relu

    with TileContext(nc) as tc:
        with (
            tc.tile_pool(name="const", bufs=1) as cpool,
            tc.tile_pool(name="x2p", bufs=2) as x2pool,
            tc.tile_pool(name="xtp", bufs=12) as xtpool,
            tc.tile_pool(name="tpp", bufs=2) as tppool,
            tc.tile_pool(name="kernp", bufs=2) as kpool,
            tc.tile_pool(name="prodp", bufs=2) as ppool,
            tc.tile_pool(name="accp", bufs=3) as apool,
            tc.tile_pool(name="ps1", bufs=2, space="PSUM") as ps1pool,
            tc.tile_pool(name="ps2", bufs=2, space="PSUM") as ps2pool,
            tc.tile_pool(name="psA", bufs=4, space="PSUM") as psApool,
        ):
            w1t = cpool.tile([C + 1, 32], BF16, tag="w1")
            w2t = cpool.tile([128, E], BF16, tag="w2")
            idt = cpool.tile([128, 128], BF16, tag="idn")
            nc.sync.dma_start(w1t[:], w1_d[:])
            nc.sync.dma_start(w2t[:], w2_d[:])
            nc.sync.dma_start(idt[:], id_d[:])

            # warmups: front-load the Act activation-table load (~1.3us)
            # and start the PE/HAM clock ramp before real work arrives
            warm = cpool.tile([128, 128], BF16, tag="warm")
            psw = ps1pool.tile([128, 512], F32, tag="ps1", name="psw")
            nc.tensor.matmul(psw[:, :128], idt[:], idt[:],
                             start=True, stop=True)
            nc.scalar.activation(warm[:], psw[:, :128], RELU)

            # block-0 x2/xt stream first, then block-1's inputs prefetch
            # behind them so no engine ever waits on an unqueued DMA
            x2tiles = [x2pool.tile([128, F2], BF16, tag="x2", name=f"x2b{i}")
                       for i in range(NB)]
            xt_pre = {}

            def _xt_fetch(blk, g):
                if (blk, g) in xt_pre:
                    return xt_pre.pop((blk, g))
                t = xtpool.tile([C + 1, 2048], BF16, tag="xt",
                                name=f"xt{blk}_{g}")
                nc.sync.dma_start(
                    t[:], xtq_d[:, blk * NPB + g * 2048:blk * NPB + (g + 1) * 2048])
                return t

            for blk in range(NB):
                st0 = blk * 128
                x2t = x2tiles[blk]

                # ---- mm1: t' = relu(x @ w1bn), 4 stripe col-groups ----
                # tp[32a+r, g*512 + (q%4)*128 + st] = relu(t')[r, q*128+st],
                # q = g*16 + a*4 + (q%4)
                tp = tppool.tile([128, 9 * 512], BF16, tag="tp")
                for g in range(9):
                    xt = _xt_fetch(blk, g)
                    ps1 = ps1pool.tile([128, 512], F32, tag="ps1")
                    # stationary is zero-padded to M=32 so each stripe
                    # fills its full 32-partition col-group (no uninit
                    # PSUM rows under the full-tile relu evac)
                    for a in range(4):
                        nc.tensor.matmul(
                            ps1[32 * a:32 * a + 32, :],
                            w1t[:],
                            xt[:, a * 512:(a + 1) * 512],
                            start=True, stop=True,
                            tile_position=(0, 32 * a))
                    nc.scalar.activation(
                        tp[:, g * 512:(g + 1) * 512], ps1[:], RELU)

                # x2 is first needed by the MAC phase; enqueue its (large)
                # DMA after the xt stream so mm1 isn't starved at startup
                nc.sync.dma_start(x2t[:], x2_d[st0:st0 + 128, :])
                if blk == 0:
                    # prefetch ALL of block 1's mm1 inputs now: they must
                    # not queue behind block-1-emitted work or the PE
                    # stalls ~10us at the block boundary
                    for g2 in range(9):
                        xt_pre[(1, g2)] = _xt_fetch(1, g2)
                    nc.sync.dma_start(
                        x2tiles[1][:], x2_d[128:256, :])

                # ---- mm2: kern[st, q*36+e] for the 144 interior q ----
                # q = 16m + 4a + r.  Matmuls sharing a ps2 bank must be in
                # the SAME PE row-group (concurrent row-groups writing one
                # bank on the same partitions are a fatal PSUM collision),
                # so group the loop by a.
                kern = kpool.tile([128, Q * E], BF16, tag="kern")
                kview = kern[:].rearrange(
                    "p (m a r e) -> p m a r e", m=9, a=4, r=4)
                # mg-outer so the first MAC chunk's kern rows complete
                # after 4 ps2 groups, not 10; consecutive groups are also
                # different row-groups AND different banks -> concurrent
                for mg in range(3):
                    for a in range(4):
                        ps2 = ps2pool.tile([128, 12 * E], F32, tag="ps2")
                        for j in range(12):
                            m, r = 3 * mg + j // 4, j % 4
                            col = m * 512 + r * 128
                            nc.tensor.matmul(
                                ps2[:, j * E:(j + 1) * E],
                                tp[32 * a:32 * a + 17, col:col + 128],
                                w2t[32 * a:32 * a + 17, :],
                                start=True, stop=True,
                                tile_position=(32 * a, 0))
                        nc.scalar.copy(
                            kview[:, 3 * mg:3 * mg + 3, a, :, :], ps2[:])

                # ---- involution MAC ----
                x2v = x2t[:].rearrange("p (h wc) -> p h wc", h=S2)     # [128,16,1024]
                kv = kern[:].rearrange("p (q e) -> p q e", e=E)        # [128,144,36]
                for ch in range(NCH):
                    i0 = 2 * ch
                    prod = ppool.tile([128, 9 * WCH], BF16, tag="prod")
                    for p in range(9):
                        di, dj = p // 3, p % 3
                        xop = x2v[:, i0 + di:i0 + di + 2,
                                  dj * C:(dj + 12) * C]                # [128,2,768]
                        krep = kv[:, 24 * ch:24 * ch + 24, 4 * p:4 * p + 4] \
                            .unsqueeze(2).broadcast_to([128, 24, CR, 4])
                        nc.vector.tensor_tensor(
                            prod[:, p * WCH:(p + 1) * WCH], xop, krep,
                            mybir.AluOpType.mult)
                        if p in (1, 3):
                            # fold a tap pair via gpsimd-triggered DMA
                            # accumulate (separate SBUF ports: steals no
                            # DVE/PE bandwidth); PE then sums 7 slots
                            nc.gpsimd.dma_start(
                                prod[:, p * WCH:(p + 1) * WCH],
                                prod[:, (p - 1) * WCH:p * WCH],
                                accum_op=mybir.AluOpType.add)
                    acct = apool.tile([128, WCH], BF16, tag="acc")
                    slot_order = [4, 5, 6, 7, 8, 1, 3]
                    for k in range(3):
                        # one PSUM bank per group, 4-deep rotation: the
                        # WAR on the scalar-engine evac trails 4 banks
                        # behind the PE stream instead of 3
                        psA = psApool.tile([128, 512], F32, tag="psA")
                        for si, p in enumerate(slot_order):
                            nc.tensor.matmul(
                                psA[:],
                                idt[:],
                                prod[:, p * WCH + k * 512:p * WCH + (k + 1) * 512],
                                start=(si == 0), stop=(si == len(slot_order) - 1))
                        nc.scalar.copy(
                            acct[:, k * 512:(k + 1) * 512], psA[:])
                    nc.sync.dma_start(
                        o_d[st0:st0 + 128, ch * WCH:(ch + 1) * WCH], acct[:])
    nc.compile()
    _CACHE["nc"] = nc
    return nc


def _host_prep(x, w1, b1, gamma, beta, mean, var, w2, b2):
    """Per-core input maps. x: [8,192,192,64] f32."""
    a = (gamma / np.sqrt(var + BN_EPS)).astype(NPF32)
    w1b = np.zeros((C + 1, 32), dtype=NPF32)
    w1b[:C, :CR] = w1 * a[None, :]
    w1b[C, :CR] = b1 * a + (beta - mean * a)
    w1b[C, CR] = 1.0                      # ones row for the b2 path
    w2r = np.zeros((128, E), dtype=NPF32)
    for r in range(4):
        w2r[32 * r:32 * r + CR] = w2
        w2r[32 * r + CR] = b2
    w1b = w1b.astype(NPBF16)
    w2r = w2r.astype(NPBF16)
    idn = np.eye(128, dtype=NPBF16)

    xb = x.astype(NPBF16)
    in_maps = []
    for b in range(B):
        xi = xb[b]
        xp2 = np.zeros((H + 2, W + 2, C), dtype=NPBF16)
        xp2[1:-1, 1:-1] = xi
        s = xp2.strides
        win2 = np.lib.stride_tricks.as_strided(
            xp2, (NG, NG, S2, S2, C), (s[0] * S, s[1] * S, s[0], s[1], s[2]))
        x2 = np.ascontiguousarray(win2).reshape(NST, F2)
        # interior pixels, q-major per block: xtq[c, blk*18432 + q*128 + st]
        arr = xi.reshape(NG, S, NG, S, C).transpose(0, 2, 1, 3, 4) \
            .reshape(NST, Q, C)
        xtq = np.empty((C + 1, NB * NPB), dtype=NPBF16)
        for blk in range(NB):
            sub = arr[blk * 128:(blk + 1) * 128]          # [128, 144, 64]
            xtq[:C, blk * NPB:(blk + 1) * NPB] = \
                sub.transpose(2, 1, 0).reshape(C, NPB)
        xtq[C] = NPBF16(1.0)
        in_maps.append({"x2": x2, "xtq": xtq, "w1b": w1b, "w2r": w2r,
                        "idn": idn})
    return in_maps


def kernel(x, w1, b1, gamma, beta, mean, var, w2, b2, _bench=None):
    nc = _build_program()
    in_maps = _host_prep(np.asarray(x), np.asarray(w1), np.asarray(b1),
                         np.asarray(gamma), np.asarray(beta), np.asarray(mean),
                         np.asarray(var), np.asarray(w2), np.asarray(b2))
    kw = dict(_bench) if _bench else {}
    res = run_bass_kernel_spmd(nc, in_maps, core_ids=list(range(B)), **kw)
    if _bench is not None:
        _bench["result"] = res
    out = np.empty((B, H, W, C), dtype=NPF32)
    for b in range(B):
        ob = res.results[b]["o"].reshape(NG, NG, S, S, C).astype(NPF32)
        out[b] = ob.transpose(0, 2, 1, 3, 4).reshape(H, W, C)
    return out


# revision 22
# speedup vs baseline: 1.1464x; 1.1464x over previous
"""Involution kernel for Trainium2, 8-core data-parallel (1 batch image per core).

Reference computation (per image, NHWC, C=64, G=4 groups, K=3, reduction 4):
    t    = relu(BN(x @ w1 + b1))            # [H,W,16]
    kern = t @ w2 + b2                      # [H,W,36], e = (ki*3+kj)*4 + g
    out[h,w,c] = sum_p kern[h,w, 4p + c%4] * xpad[h+di-1, w+dj-1, c]

Device strategy (v2):
  * 256 subtiles of 12x12 interior, each in one SBUF partition's free dim
    with a 2-ring halo (16x16x64 window, x2 layout).  3x3 taps are free-dim
    offsets.  2 blocks of 128 subtiles.
  * kern is computed ONLY for the 144 interior positions per subtile.
  * mm1 (x @ w1bn) runs as 4 concurrent column-group matmuls
    (tile_position stripes at PSUM partitions 0/32/64/96, 17 rows each:
    16 bottleneck channels + a ones row for the b2 path), on a q-major
    pixel layout (col = q*128 + st).  Scalar engine applies ReLU while
    evacuating [128,512] PSUM tiles -> tp.
  * mm2: per-position matmul lhsT=tp-slab [17,128st], rhs=w2 replica at the
    matching partition base; out [128st, 36].  Scalar engine casts to bf16.
  * Involution: DVE computes the 9 per-tap products (bf16, 2x mode);
    the 8 adds run on the TENSOR engine as identity matmuls accumulating
    in PSUM (9 matmuls per 512-col bank group).  Scalar engine evacuates
    f32 PSUM -> bf16 out tile; DMA writes interior layout directly.
"""

import numpy as np
import ml_dtypes

import concourse.bass as bass
import concourse.bacc as bacc
import concourse.mybir as mybir
from concourse.tile import TileContext
from concourse.bass_utils import run_bass_kernel_spmd

BF16 = mybir.dt.bfloat16
F32 = mybir.dt.float32
NPF32 = np.float32
NPBF16 = ml_dtypes.bfloat16

B, H, W, C = 8, 192, 192, 64
G, K, CR, E = 4, 3, 16, 36
BN_EPS = 1e-3
S = 12                  # subtile interior
S2 = S + 2              # 1-ring padded subtile size (14)
NG = H // S             # 16 subtiles per axis
NST = NG * NG           # 256 subtiles
NB = 2                  # partition blocks of 128 subtiles
Q = S * S               # 144 interior positions per subtile
F2 = S2 * S2 * C        # 16384 x2 free elems per subtile
FO = Q * C              # 9216 out free elems per subtile
NPB = 128 * Q           # 18432 pixels per block
WCH = 1536              # MAC chunk width (2 interior rows = 24 q)
NCH = FO // WCH         # 6 chunks per block

_CACHE = {}


def _build_program():
    if "nc" in _CACHE:
        return _CACHE["nc"]
    nc = bacc.Bacc(None, target_bir_lowering=False)
    x2_d = nc.dram_tensor("x2", [NST, F2], BF16, kind="ExternalInput")
    xtq_d = nc.dram_tensor("xtq", [C + 1, NB * NPB], BF16, kind="ExternalInput")
    w1_d = nc.dram_tensor("w1b", [C + 1, 32], BF16, kind="ExternalInput")
    w2_d = nc.dram_tensor("w2r", [128, E], BF16, kind="ExternalInput")
    id_d = nc.dram_tensor("idn", [128, 128], BF16, kind="ExternalInput")
    o_d = nc.dram_tensor("o", [NST, FO], BF16, kind="ExternalOutput")

    RELU = mybir.ActivationFunctionType.Relu

    with TileContext(nc) as tc:
        with (
            tc.tile_pool(name="const", bufs=1) as cpool,
            tc.tile_pool(name="x2p", bufs=2) as x2pool,
            tc.tile_pool(name="xtp", bufs=12) as xtpool,
            tc.tile_pool(name="tpp", bufs=2) as tppool,
            tc.tile_pool(name="kernp", bufs=2) as kpool,
            tc.tile_pool(name="prodp", bufs=2) as ppool,
            tc.tile_pool(name="accp", bufs=3) as apool,
            tc.tile_pool(name="ps1", bufs=2, space="PSUM") as ps1pool,
            tc.tile_pool(name="ps2", bufs=2, space="PSUM") as ps2pool,
            tc.tile_pool(name="psA", bufs=4, space="PSUM") as psApool,
        ):
            w1t = cpool.tile([C + 1, 32], BF16, tag="w1")
            w2t = cpool.tile([128, E], BF16, tag="w2")
            idt = cpool.tile([128, 128], BF16, tag="idn")
            nc.sync.dma_start(w1t[:], w1_d[:])
            nc.sync.dma_start(w2t[:], w2_d[:])
            nc.sync.dma_start(idt[:], id_d[:])

            # warmups: front-load the Act activation-table load (~1.3us)
            # and start the PE/HAM clock ramp before real work arrives
            warm = cpool.tile([128, 128], BF16, tag="warm")
            psw = ps1pool.tile([128, 512], F32, tag="ps1", name="psw")
            nc.tensor.matmul(psw[:, :128], idt[:], idt[:],
                             start=True, stop=True)
            nc.scalar.activation(warm[:], psw[:, :128], RELU)

            # block-0 x2/xt stream first, then block-1's inputs prefetch
            # behind them so no engine ever waits on an unqueued DMA
            x2tiles = [x2pool.tile([128, F2], BF16, tag="x2", name=f"x2b{i}")
                       for i in range(NB)]
            xt_pre = {}

            def _xt_fetch(blk, g):
                if (blk, g) in xt_pre:
                    return xt_pre.pop((blk, g))
                t = xtpool.tile([C + 1, 2048], BF16, tag="xt",
                                name=f"xt{blk}_{g}")
                nc.sync.dma_start(
                    t[:], xtq_d[:, blk * NPB + g * 2048:blk * NPB + (g + 1) * 2048])
                return t

            for blk in range(NB):
                st0 = blk * 128
                x2t = x2tiles[blk]

                # ---- mm1: t' = relu(x @ w1bn), 4 stripe col-groups ----
                # tp[32a+r, g*512 + (q%4)*128 + st] = relu(t')[r, q*128+st],
                # q = g*16 + a*4 + (q%4)
                tp = tppool.tile([128, 9 * 512], BF16, tag="tp")
                for g in range(9):
                    xt = _xt_fetch(blk, g)
                    ps1 = ps1pool.tile([128, 512], F32, tag="ps1")
                    # stationary is zero-padded to M=32 so each stripe
                    # fills its full 32-partition col-group (no uninit
                    # PSUM rows under the full-tile relu evac)
                    for a in range(4):
                        nc.tensor.matmul(
                            ps1[32 * a:32 * a + 32, :],
                            w1t[:],
                            xt[:, a * 512:(a + 1) * 512],
                            start=True, stop=True,
                            tile_position=(0, 32 * a))
                    nc.scalar.activation(
                        tp[:, g * 512:(g + 1) * 512], ps1[:], RELU)

                # x2 is first needed by the MAC phase; enqueue its (large)
                # DMA after the xt stream so mm1 isn't starved at startup
                nc.sync.dma_start(x2t[:], x2_d[st0:st0 + 128, :])
                if blk == 0:
                    # prefetch ALL of block 1's mm1 inputs now: they must
                    # not queue behind block-1-emitted work or the PE
                    # stalls ~10us at the block boundary
                    for g2 in range(9):
                        xt_pre[(1, g2)] = _xt_fetch(1, g2)
                    nc.sync.dma_start(
                        x2tiles[1][:], x2_d[128:256, :])

                # ---- mm2: kern[st, q*36+e] for the 144 interior q ----
                # q = 16m + 4a + r.  Matmuls sharing a ps2 bank must be in
                # the SAME PE row-group (concurrent row-groups writing one
                # bank on the same partitions are a fatal PSUM collision),
                # so group the loop by a.
                kern = kpool.tile([128, Q * E], BF16, tag="kern")
                kview = kern[:].rearrange(
                    "p (m a r e) -> p m a r e", m=9, a=4, r=4)
                # mg-outer so the first MAC chunk's kern rows complete
                # after 4 ps2 groups, not 10; consecutive groups are also
                # different row-groups AND different banks -> concurrent
                for mg in range(3):
                    for a in range(4):
                        ps2 = ps2pool.tile([128, 12 * E], F32, tag="ps2")
                        for j in range(12):
                            m, r = 3 * mg + j // 4, j % 4
                            col = m * 512 + r * 128
                            nc.tensor.matmul(
                                ps2[:, j * E:(j + 1) * E],
                                tp[32 * a:32 * a + 17, col:col + 128],
                                w2t[32 * a:32 * a + 17, :],
                                start=True, stop=True,
                                tile_position=(32 * a, 0))
                        nc.scalar.copy(
                            kview[:, 3 * mg:3 * mg + 3, a, :, :], ps2[:])

                # ---- involution MAC ----
                x2v = x2t[:].rearrange("p (h wc) -> p h wc", h=S2)     # [128,16,1024]
                kv = kern[:].rearrange("p (q e) -> p q e", e=E)        # [128,144,36]
                for ch in range(NCH):
                    i0 = 2 * ch
                    prod = ppool.tile([128, 9 * WCH], BF16, tag="prod")
                    for p in range(9):
                        di, dj = p // 3, p % 3
                        xop = x2v[:, i0 + di:i0 + di + 2,
                                  dj * C:(dj + 12) * C]                # [128,2,768]
                        krep = kv[:, 24 * ch:24 * ch + 24, 4 * p:4 * p + 4] \
                            .unsqueeze(2).broadcast_to([128, 24, CR, 4])
                        nc.vector.tensor_tensor(
                            prod[:, p * WCH:(p + 1) * WCH], xop, krep,
                            mybir.AluOpType.mult)
                    acct = apool.tile([128, WCH], BF16, tag="acc")
                    for k in range(3):
                        # one PSUM bank per group, 4-deep rotation: the
                        # WAR on the scalar-engine evac trails 4 banks
                        # behind the PE stream instead of 3
                        psA = psApool.tile([128, 512], F32, tag="psA")
                        for p in range(9):
                            nc.tensor.matmul(
                                psA[:],
                                idt[:],
                                prod[:, p * WCH + k * 512:p * WCH + (k + 1) * 512],
                                start=(p == 0), stop=(p == 8))
                        nc.scalar.copy(
                            acct[:, k * 512:(k + 1) * 512], psA[:])
                    nc.sync.dma_start(
                        o_d[st0:st0 + 128, ch * WCH:(ch + 1) * WCH], acct[:])
    nc.compile()
    _CACHE["nc"] = nc
    return nc


def _host_prep(x, w1, b1, gamma, beta, mean, var, w2, b2):
    """Per-core input maps. x: [8,192,192,64] f32."""
    a = (gamma / np.sqrt(var + BN_EPS)).astype(NPF32)
    w1b = np.zeros((C + 1, 32), dtype=NPF32)
    w1b[:C, :CR] = w1 * a[None, :]
    w1b[C, :CR] = b1 * a + (beta - mean * a)
    w1b[C, CR] = 1.0                      # ones row for the b2 path
    w2r = np.zeros((128, E), dtype=NPF32)
    for r in range(4):
        w2r[32 * r:32 * r + CR] = w2
        w2r[32 * r + CR] = b2
    w1b = w1b.astype(NPBF16)
    w2r = w2r.astype(NPBF16)
    idn = np.eye(128, dtype=NPBF16)

    xb = x.astype(NPBF16)
    in_maps = []
    for b in range(B):
        xi = xb[b]
        xp2 = np.zeros((H + 2, W + 2, C), dtype=NPBF16)
        xp2[1:-1, 1:-1] = xi
        s = xp2.strides
        win2 = np.lib.stride_tricks.as_strided(
            xp2, (NG, NG, S2, S2, C), (s[0] * S, s[1] * S, s[0], s[1], s[2]))
        x2 = np.ascontiguousarray(win2).reshape(NST, F2)
        # interior pixels, q-major per block: xtq[c, blk*18432 + q*128 + st]
        arr = xi.reshape(NG, S, NG, S, C).transpose(0, 2, 1, 3, 4) \
            .reshape(NST, Q, C)
        xtq = np.empty((C + 1, NB * NPB), dtype=NPBF16)
        for blk in range(NB):
            sub = arr[blk * 128:(blk + 1) * 128]          # [128, 144, 64]
            xtq[:C, blk * NPB:(blk + 1) * NPB] = \
                sub.transpose(2, 1, 0).reshape(C, NPB)
        xtq[C] = NPBF16(1.0)
        in_maps.append({"x2": x2, "xtq": xtq, "w1b": w1b, "w2r": w2r,
                        "idn": idn})
    return in_maps


def kernel(x, w1, b1, gamma, beta, mean, var, w2, b2, _bench=None):
    nc = _build_program()
    in_maps = _host_prep(np.asarray(x), np.asarray(w1), np.asarray(b1),
                         np.asarray(gamma), np.asarray(beta), np.asarray(mean),
                         np.asarray(var), np.asarray(w2), np.asarray(b2))
    kw = dict(_bench) if _bench else {}
    res = run_bass_kernel_spmd(nc, in_maps, core_ids=list(range(B)), **kw)
    if _bench is not None:
        _bench["result"] = res
    out = np.empty((B, H, W, C), dtype=NPF32)
    for b in range(B):
        ob = res.results[b]["o"].reshape(NG, NG, S, S, C).astype(NPF32)
        out[b] = ob.transpose(0, 2, 1, 3, 4).reshape(H, W, C)
    return out


# revision 23
# speedup vs baseline: 1.1592x; 1.0111x over previous
"""Involution kernel for Trainium2, 8-core data-parallel (1 batch image per core).

Reference computation (per image, NHWC, C=64, G=4 groups, K=3, reduction 4):
    t    = relu(BN(x @ w1 + b1))            # [H,W,16]
    kern = t @ w2 + b2                      # [H,W,36], e = (ki*3+kj)*4 + g
    out[h,w,c] = sum_p kern[h,w, 4p + c%4] * xpad[h+di-1, w+dj-1, c]

Device strategy (v2):
  * 256 subtiles of 12x12 interior, each in one SBUF partition's free dim
    with a 2-ring halo (16x16x64 window, x2 layout).  3x3 taps are free-dim
    offsets.  2 blocks of 128 subtiles.
  * kern is computed ONLY for the 144 interior positions per subtile.
  * mm1 (x @ w1bn) runs as 4 concurrent column-group matmuls
    (tile_position stripes at PSUM partitions 0/32/64/96, 17 rows each:
    16 bottleneck channels + a ones row for the b2 path), on a q-major
    pixel layout (col = q*128 + st).  Scalar engine applies ReLU while
    evacuating [128,512] PSUM tiles -> tp.
  * mm2: per-position matmul lhsT=tp-slab [17,128st], rhs=w2 replica at the
    matching partition base; out [128st, 36].  Scalar engine casts to bf16.
  * Involution: DVE computes the 9 per-tap products (bf16, 2x mode);
    the 8 adds run on the TENSOR engine as identity matmuls accumulating
    in PSUM (9 matmuls per 512-col bank group).  Scalar engine evacuates
    f32 PSUM -> bf16 out tile; DMA writes interior layout directly.
"""

import numpy as np
import ml_dtypes

import concourse.bass as bass
import concourse.bacc as bacc
import concourse.mybir as mybir
from concourse.tile import TileContext
from concourse.bass_utils import run_bass_kernel_spmd

BF16 = mybir.dt.bfloat16
F32 = mybir.dt.float32
NPF32 = np.float32
NPBF16 = ml_dtypes.bfloat16

B, H, W, C = 8, 192, 192, 64
G, K, CR, E = 4, 3, 16, 36
BN_EPS = 1e-3
S = 12                  # subtile interior
S2 = S + 2              # 1-ring padded subtile size (14)
NG = H // S             # 16 subtiles per axis
NST = NG * NG           # 256 subtiles
NB = 2                  # partition blocks of 128 subtiles
Q = S * S               # 144 interior positions per subtile
F2 = S2 * S2 * C        # 16384 x2 free elems per subtile
FO = Q * C              # 9216 out free elems per subtile
NPB = 128 * Q           # 18432 pixels per block
WCH = 1536              # MAC chunk width (2 interior rows = 24 q)
NCH = FO // WCH         # 6 chunks per block

_CACHE = {}


def _build_program():
    if "nc" in _CACHE:
        return _CACHE["nc"]
    nc = bacc.Bacc(None, target_bir_lowering=False)
    x2_d = nc.dram_tensor("x2", [NST, F2], BF16, kind="ExternalInput")
    xtq_d = nc.dram_tensor("xtq", [C + 1, NB * NPB], BF16, kind="ExternalInput")
    w1_d = nc.dram_tensor("w1b", [C + 1, 32], BF16, kind="ExternalInput")
    w2_d = nc.dram_tensor("w2r", [128, E], BF16, kind="ExternalInput")
    id_d = nc.dram_tensor("idn", [128, 128], BF16, kind="ExternalInput")
    o_d = nc.dram_tensor("o", [NST, FO], BF16, kind="ExternalOutput")

    RELU = mybir.ActivationFunctionType.Relu

    with TileContext(nc) as tc:
        with (
            tc.tile_pool(name="const", bufs=1) as cpool,
            tc.tile_pool(name="x2p", bufs=2) as x2pool,
            tc.tile_pool(name="xtp", bufs=12) as xtpool,
            tc.tile_pool(name="tpp", bufs=2) as tppool,
            tc.tile_pool(name="kernp", bufs=2) as kpool,
            tc.tile_pool(name="prodp", bufs=2) as ppool,
            tc.tile_pool(name="accp", bufs=3) as apool,
            tc.tile_pool(name="ps1", bufs=2, space="PSUM") as ps1pool,
            tc.tile_pool(name="ps2", bufs=2, space="PSUM") as ps2pool,
            tc.tile_pool(name="psA", bufs=4, space="PSUM") as psApool,
        ):
            w1t = cpool.tile([C + 1, 32], BF16, tag="w1")
            w2t = cpool.tile([128, E], BF16, tag="w2")
            idt = cpool.tile([128, 128], BF16, tag="idn")
            nc.sync.dma_start(w1t[:], w1_d[:])
            nc.sync.dma_start(w2t[:], w2_d[:])
            nc.sync.dma_start(idt[:], id_d[:])

            # warmups: front-load the Act activation-table load (~1.3us)
            # and start the PE/HAM clock ramp before real work arrives
            warm = cpool.tile([128, 128], BF16, tag="warm")
            psw = ps1pool.tile([128, 512], F32, tag="ps1", name="psw")
            nc.tensor.matmul(psw[:, :128], idt[:], idt[:],
                             start=True, stop=True)
            nc.scalar.activation(warm[:], psw[:, :128], RELU)

            # block-0 x2/xt stream first, then block-1's inputs prefetch
            # behind them so no engine ever waits on an unqueued DMA
            x2tiles = [x2pool.tile([128, F2], BF16, tag="x2", name=f"x2b{i}")
                       for i in range(NB)]
            xt_pre = {}

            def _xt_fetch(blk, g):
                if (blk, g) in xt_pre:
                    return xt_pre.pop((blk, g))
                t = xtpool.tile([C + 1, 2048], BF16, tag="xt",
                                name=f"xt{blk}_{g}")
                nc.sync.dma_start(
                    t[:], xtq_d[:, blk * NPB + g * 2048:blk * NPB + (g + 1) * 2048])
                return t

            for blk in range(NB):
                st0 = blk * 128
                x2t = x2tiles[blk]

                # ---- mm1 + mm2 interleaved ----
                # mm1: t' = relu(x @ w1bn), 4 stripe col-groups:
                # tp[32a+r, g*512 + (q%4)*128 + st] = relu(t')[r, q*128+st],
                # q = g*16 + a*4 + (q%4).
                # mm2 group mg reads only tp cols from mm1 groups
                # g = 3mg..3mg+2, so emit each mm2 mg right after those —
                # the first MAC chunk's kern is ready ~10us of PE-work
                # earlier than with phase-sequential emission.
                # (q = 16m + 4a + r; matmuls sharing a ps2 bank stay in
                # the SAME PE row-group — concurrent row-groups writing
                # one bank on the same partitions are a fatal collision.)
                tp = tppool.tile([128, 9 * 512], BF16, tag="tp")
                kern = kpool.tile([128, Q * E], BF16, tag="kern")
                kview = kern[:].rearrange(
                    "p (m a r e) -> p m a r e", m=9, a=4, r=4)
                for mg in range(3):
                    for gg in range(3):
                        g = 3 * mg + gg
                        xt = _xt_fetch(blk, g)
                        ps1 = ps1pool.tile([128, 512], F32, tag="ps1")
                        # stationary is zero-padded to M=32 so each stripe
                        # fills its full 32-partition col-group (no uninit
                        # PSUM rows under the full-tile relu evac)
                        for a in range(4):
                            nc.tensor.matmul(
                                ps1[32 * a:32 * a + 32, :],
                                w1t[:],
                                xt[:, a * 512:(a + 1) * 512],
                                start=True, stop=True,
                                tile_position=(0, 32 * a))
                        nc.scalar.activation(
                            tp[:, g * 512:(g + 1) * 512], ps1[:], RELU)
                    for a in range(4):
                        ps2 = ps2pool.tile([128, 12 * E], F32, tag="ps2")
                        for j in range(12):
                            m, r = 3 * mg + j // 4, j % 4
                            col = m * 512 + r * 128
                            nc.tensor.matmul(
                                ps2[:, j * E:(j + 1) * E],
                                tp[32 * a:32 * a + 17, col:col + 128],
                                w2t[32 * a:32 * a + 17, :],
                                start=True, stop=True,
                                tile_position=(32 * a, 0))
                        nc.scalar.copy(
                            kview[:, 3 * mg:3 * mg + 3, a, :, :], ps2[:])

                # x2 is first needed by the MAC phase; enqueue its (large)
                # DMA after the xt stream so mm1 isn't starved at startup
                nc.sync.dma_start(x2t[:], x2_d[st0:st0 + 128, :])
                if blk == 0:
                    # prefetch ALL of block 1's mm1 inputs now: they must
                    # not queue behind block-1-emitted work or the PE
                    # stalls ~10us at the block boundary
                    for g2 in range(9):
                        xt_pre[(1, g2)] = _xt_fetch(1, g2)
                    nc.sync.dma_start(
                        x2tiles[1][:], x2_d[128:256, :])

                # ---- involution MAC ----
                x2v = x2t[:].rearrange("p (h wc) -> p h wc", h=S2)     # [128,16,1024]
                kv = kern[:].rearrange("p (q e) -> p q e", e=E)        # [128,144,36]
                for ch in range(NCH):
                    i0 = 2 * ch
                    prod = ppool.tile([128, 9 * WCH], BF16, tag="prod")
                    for p in range(9):
                        di, dj = p // 3, p % 3
                        xop = x2v[:, i0 + di:i0 + di + 2,
                                  dj * C:(dj + 12) * C]                # [128,2,768]
                        krep = kv[:, 24 * ch:24 * ch + 24, 4 * p:4 * p + 4] \
                            .unsqueeze(2).broadcast_to([128, 24, CR, 4])
                        nc.vector.tensor_tensor(
                            prod[:, p * WCH:(p + 1) * WCH], xop, krep,
                            mybir.AluOpType.mult)
                    acct = apool.tile([128, WCH], BF16, tag="acc")
                    for k in range(3):
                        # one PSUM bank per group, 4-deep rotation: the
                        # WAR on the scalar-engine evac trails 4 banks
                        # behind the PE stream instead of 3
                        psA = psApool.tile([128, 512], F32, tag="psA")
                        for p in range(9):
                            nc.tensor.matmul(
                                psA[:],
                                idt[:],
                                prod[:, p * WCH + k * 512:p * WCH + (k + 1) * 512],
                                start=(p == 0), stop=(p == 8))
                        nc.scalar.copy(
                            acct[:, k * 512:(k + 1) * 512], psA[:])
                    nc.sync.dma_start(
                        o_d[st0:st0 + 128, ch * WCH:(ch + 1) * WCH], acct[:])
    nc.compile()
    _CACHE["nc"] = nc
    return nc


def _host_prep(x, w1, b1, gamma, beta, mean, var, w2, b2):
    """Per-core input maps. x: [8,192,192,64] f32."""
    a = (gamma / np.sqrt(var + BN_EPS)).astype(NPF32)
    w1b = np.zeros((C + 1, 32), dtype=NPF32)
    w1b[:C, :CR] = w1 * a[None, :]
    w1b[C, :CR] = b1 * a + (beta - mean * a)
    w1b[C, CR] = 1.0                      # ones row for the b2 path
    w2r = np.zeros((128, E), dtype=NPF32)
    for r in range(4):
        w2r[32 * r:32 * r + CR] = w2
        w2r[32 * r + CR] = b2
    w1b = w1b.astype(NPBF16)
    w2r = w2r.astype(NPBF16)
    idn = np.eye(128, dtype=NPBF16)

    xb = x.astype(NPBF16)
    in_maps = []
    for b in range(B):
        xi = xb[b]
        xp2 = np.zeros((H + 2, W + 2, C), dtype=NPBF16)
        xp2[1:-1, 1:-1] = xi
        s = xp2.strides
        win2 = np.lib.stride_tricks.as_strided(
            xp2, (NG, NG, S2, S2, C), (s[0] * S, s[1] * S, s[0], s[1], s[2]))
        x2 = np.ascontiguousarray(win2).reshape(NST, F2)
        # interior pixels, q-major per block: xtq[c, blk*18432 + q*128 + st]
        arr = xi.reshape(NG, S, NG, S, C).transpose(0, 2, 1, 3, 4) \
            .reshape(NST, Q, C)
        xtq = np.empty((C + 1, NB * NPB), dtype=NPBF16)
        for blk in range(NB):
            sub = arr[blk * 128:(blk + 1) * 128]          # [128, 144, 64]
            xtq[:C, blk * NPB:(blk + 1) * NPB] = \
                sub.transpose(2, 1, 0).reshape(C, NPB)
        xtq[C] = NPBF16(1.0)
        in_maps.append({"x2": x2, "xtq": xtq, "w1b": w1b, "w2r": w2r,
                        "idn": idn})
    return in_maps


def kernel(x, w1, b1, gamma, beta, mean, var, w2, b2, _bench=None):
    nc = _build_program()
    in_maps = _host_prep(np.asarray(x), np.asarray(w1), np.asarray(b1),
                         np.asarray(gamma), np.asarray(beta), np.asarray(mean),
                         np.asarray(var), np.asarray(w2), np.asarray(b2))
    kw = dict(_bench) if _bench else {}
    res = run_bass_kernel_spmd(nc, in_maps, core_ids=list(range(B)), **kw)
    if _bench is not None:
        _bench["result"] = res
    out = np.empty((B, H, W, C), dtype=NPF32)
    for b in range(B):
        ob = res.results[b]["o"].reshape(NG, NG, S, S, C).astype(NPF32)
        out[b] = ob.transpose(0, 2, 1, 3, 4).reshape(H, W, C)
    return out
